# revision 10
# baseline (speedup 1.0000x reference)
"""Trainium2 Bass kernel for nn_Attention_73254962200646.

Reference computation (per batch element b, all shapes hardcoded):
  qkv = conv3x3(x, W_qkv, pad=1)            x:[8,512,32,32], W_qkv:[1536,512,3,3]
  q,k,v -> [g=8 heads, n=1024, d=64]
  attn  = (q @ k^T) / (|q| |k| + eps)       cosine-similarity attention
  out   = attn @ v -> [512,32,32]
  out   = conv1x1(out, W_out); BatchNorm2d (batch stats); ReLU

Distribution: data-parallel over batch B=8 across the 8 NeuronCores (one
image per core). All compute is core-local in bf16 (fp32 PSUM accumulation);
the only collective is a 4KB AllReduce of the BatchNorm partial sums.

The emission order interleaves phases at ~2us granularity so the PE never
idles waiting on PSUM->SBUF copies (which would also drop the HAM clock
from 2.4 to 1.2 GHz): each block's epilogue (copies, norms, transposes)
is woven into the next block's conv matmuls, and each attention pair is
woven into the following conv block.
"""

import numpy as np
import ml_dtypes

import concourse.tile as tile
import concourse.mybir as mybir
from concourse import bacc, bass_utils

BF = ml_dtypes.bfloat16
SMOOTH = 1e-4
BN_EPS = 1e-5
NCORES = 8

_NC = None
LAST_RESULT = None


def _build():
    f32 = mybir.dt.float32
    bf = mybir.dt.bfloat16
    AF = mybir.ActivationFunctionType
    ALU = mybir.AluOpType

    nc = bacc.Bacc("TRN2", target_bir_lowering=False, debug=False,
                   num_devices=NCORES)
    xin = nc.dram_tensor("xpad", [4, 128, 34, 34], bf, kind="ExternalInput").ap()
    wqk = nc.dram_tensor("wqk", [12, 4, 128, 3, 3, 128], bf, kind="ExternalInput").ap()
    wo = nc.dram_tensor("wo", [4, 128, 512], bf, kind="ExternalInput").ap()
    gb = nc.dram_tensor("gb", [128, 8], f32, kind="ExternalInput").ap()
    ones2 = nc.dram_tensor("ones2", [128, 2], bf, kind="ExternalInput").ap()
    sel2 = nc.dram_tensor("sel2", [2, 128], f32, kind="ExternalInput").ap()
    ident = nc.dram_tensor("ident", [128, 128], bf, kind="ExternalInput").ap()
    out = nc.dram_tensor("out", [512, 1024], f32, kind="ExternalOutput").ap()

    with tile.TileContext(nc) as tc:
        with tc.tile_pool(name="sb", bufs=1) as sb, \
             tc.tile_pool(name="tp", bufs=2) as tp, \
             tc.tile_pool(name="ps", bufs=4, space="PSUM") as ps, \
             tc.tile_pool(name="dram", bufs=1, space="DRAM") as dram:

            xps = [sb.tile([128, 34, 34], bf, tag=f"xp{cb}", name=f"xp{cb}")
                   for cb in range(4)]
            identt = sb.tile([128, 128], bf, tag="identt")
            wot = sb.tile([128, 4, 512], bf, tag="wot")
            gbt = sb.tile([128, 8], f32, tag="gbt")
            ones2t = sb.tile([128, 2], bf, tag="ones2t")
            sel2t = sb.tile([2, 128], f32, tag="sel2t")
            qhat = sb.tile([128, 4, 1024], bf, tag="qhat")
            khat = sb.tile([128, 4, 1024], bf, tag="khat")
            vT = sb.tile([128, 8, 512], bf, tag="vT")
            att = sb.tile([128, 4, 1024], bf, tag="att")
            yt = sb.tile([128, 4, 1024], f32, tag="yt")
            part = sb.tile([128, 8], f32, tag="part")
            stats = sb.tile([128, 8], f32, tag="stats")
            epst = sb.tile([128, 1], f32, tag="epst")
            smt = sb.tile([2, 1], f32, tag="smt")

            # startup DMAs: sync queue is reserved for the weight stream
            # (the first conv chunk needs wqk[8,0] + xp[0] as early as
            # possible), so the four input panes go out on four other queues
            # in parallel and everything needed later trails behind them.
            nc.scalar.dma_start(xps[0][:], xin[0])
            nc.gpsimd.dma_start(xps[1][:], xin[1])
            nc.scalar.dma_start(xps[2][:], xin[2])
            nc.gpsimd.dma_start(xps[3][:], xin[3])
            nc.gpsimd.dma_start(identt[:], ident)
            nc.gpsimd.dma_start(ones2t[:], ones2)
            nc.gpsimd.dma_start(sel2t[:], sel2)
            for cb in range(4):
                nc.gpsimd.dma_start(wot[:, cb], wo[cb])
            nc.gpsimd.dma_start(gbt[:], gb)
            nc.vector.memset(epst[:], BN_EPS)
            nc.vector.memset(smt[:], SMOOTH)

            def emit_warm_ar():
                # tiny warm-up AllReduce: pays the ncfw cold-entry cost while
                # the convs run, so the tail BN AllReduce enters a warm path
                warm_in = dram.tile([1, 8], f32, name="warm_in")
                warm_out = dram.tile([1, 8], f32, name="warm_out")
                warm_sb = sb.tile([1, 8], f32, tag="warm_sb")
                nc.vector.memset(warm_sb[:], 0.0)
                nc.gpsimd.dma_start(warm_in[:], warm_sb[:])
                nc.gpsimd.collective_compute(
                    "AllReduce", ALU.add,
                    ins=[warm_in[:].opt()], outs=[warm_out[:].opt()],
                    replica_groups=[list(range(NCORES))])

            def conv_gen(cob):
                """Yields (pq, raw) after DMA issue, then None per 9-MM chunk."""
                wqts = [tp.tile([128, 3, 3, 128], bf, tag=f"wq{cb}", bufs=3,
                                name=f"wqt{cob}_{cb}") for cb in range(4)]
                for cb in range(4):
                    nc.sync.dma_start(wqts[cb][:], wqk[cob, cb])
                pq = ps.tile([128, 1024], f32, tag="mmp", bufs=4,
                             name=f"pq{cob}")
                raw = tp.tile([128, 1024], bf, tag="raw", bufs=4,
                              name=f"raw{cob}")
                yield (pq, raw)
                for t in range(2):
                    k = 0
                    for cb in range(4):
                        for ky in range(3):
                            for kx in range(3):
                                nc.tensor.matmul(
                                    pq[:, 512 * t:512 * (t + 1)],
                                    wqts[cb][:, ky, kx, :],
                                    xps[cb][:, 16 * t + ky:16 * t + ky + 16,
                                            kx:kx + 32],
                                    start=(k == 0), stop=(k == 35))
                                k += 1
                        yield None

            def post_gen(cob, pq, raw):
                """Epilogue for a conv block: psum copy, then per-kind tail."""
                nc.scalar.copy(raw[:, 0:512], pq[:, 0:512])
                nc.vector.tensor_copy(out=raw[:, 512:1024], in_=pq[:, 512:1024])
                yield None
                if cob >= 8:   # v block: PE-transpose into vT
                    m = cob - 8
                    for c2 in range(2):
                        pt = ps.tile([128, 512], bf, tag="mmp", bufs=4,
                                     name=f"pt{cob}_{c2}")
                        for c in range(4):
                            j = 4 * c2 + c
                            nc.tensor.transpose(pt[:, 128 * c:128 * (c + 1)],
                                                raw[:, 128 * j:128 * (j + 1)],
                                                identt[:])
                        dstv = vT[:, 4 * c2:4 * (c2 + 1), 128 * m:128 * (m + 1)]
                        srcv = pt[:].rearrange("p (a b) -> p a b", a=4)
                        if c2 == 0:
                            nc.scalar.copy(dstv, srcv)
                        else:
                            nc.vector.tensor_copy(out=dstv, in_=srcv)
                        yield None
                else:          # q/k block: cosine norms + normalized copy
                    m = cob % 4
                    dst = qhat if cob < 4 else khat
                    nrm = tp.tile([2, 1024], f32, tag="nrm", bufs=2,
                                  name=f"nrm{cob}")
                    inv = tp.tile([2, 1024], f32, tag="inv", bufs=2,
                                  name=f"inv{cob}")
                    sq = tp.tile([128, 1024], bf, tag="sq", bufs=2,
                                 name=f"sq{cob}")
                    nc.scalar.square(sq[:, 0:512], raw[:, 0:512])
                    nc.vector.tensor_mul(sq[:, 512:1024], raw[:, 512:1024],
                                         raw[:, 512:1024])
                    yield None
                    for t in range(2):
                        pss = ps.tile([2, 512], f32, tag="mmp", bufs=4,
                                      name=f"pss{cob}_{t}")
                        nc.tensor.matmul(pss[:], ones2t[:],
                                         sq[:, 512 * t:512 * (t + 1)],
                                         start=True, stop=True)
                        nc.scalar.activation(out=nrm[:, 512 * t:512 * (t + 1)],
                                             in_=pss[:], func=AF.Sqrt,
                                             bias=smt[:], scale=1.0)
                        yield None
                    nc.vector.reciprocal_approx_fast(out=inv[:], in_=nrm[:])
                    yield None
                    for t in range(2):
                        pbc = ps.tile([128, 512], f32, tag="mmp", bufs=4,
                                      name=f"pbc{cob}_{t}")
                        nc.tensor.matmul(pbc[:], sel2t[:],
                                         inv[:, 512 * t:512 * (t + 1)],
                                         start=True, stop=True)
                        nc.vector.tensor_mul(dst[:, m, 512 * t:512 * (t + 1)],
                                             raw[:, 512 * t:512 * (t + 1)],
                                             pbc[:])
                        yield None

            def att_gen(m):
                """Attention pair (heads 2m, 2m+1): 2 chunks per j block."""
                po = ps.tile([128, 1024], f32, tag="mmp", bufs=4, name=f"po{m}")
                prev = None
                for j in range(8):
                    if prev is not None:
                        emit_outT(m, po, *prev)
                    pa0 = ps.tile([128, 1024], f32, tag="mmp", bufs=4,
                                  name=f"pa0_{m}_{j}")
                    pa1 = ps.tile([128, 1024], f32, tag="mmp", bufs=4,
                                  name=f"pa1_{m}_{j}")
                    for t in range(2):
                        nc.tensor.matmul(pa0[:, 512 * t:512 * (t + 1)],
                                         khat[0:64, m, 128 * j:128 * (j + 1)],
                                         qhat[0:64, m, 512 * t:512 * (t + 1)],
                                         start=True, stop=True)
                        nc.tensor.matmul(pa1[:, 512 * t:512 * (t + 1)],
                                         khat[64:128, m, 128 * j:128 * (j + 1)],
                                         qhat[64:128, m, 512 * t:512 * (t + 1)],
                                         start=True, stop=True)
                    a0 = tp.tile([128, 1024], bf, tag="attnT", bufs=6,
                                 name=f"a0_{m}_{j}")
                    a1 = tp.tile([128, 1024], bf, tag="attnT", bufs=6,
                                 name=f"a1_{m}_{j}")
                    nc.scalar.copy(a0[:], pa0[:])
                    nc.vector.tensor_copy(out=a1[:], in_=pa1[:])
                    prev = (j, a0, a1)
                    yield None
                emit_outT(m, po, *prev)
                if m % 2 == 0:
                    nc.scalar.copy(att[:, m, :], po[:])
                else:
                    nc.vector.tensor_copy(out=att[:, m, :], in_=po[:])
                yield None

            def emit_outT(m, po, j, a0, a1):
                for t in range(2):
                    nc.tensor.matmul(po[0:64, 512 * t:512 * (t + 1)],
                                     vT[:, j, 128 * m:128 * m + 64],
                                     a0[:, 512 * t:512 * (t + 1)],
                                     start=(j == 0), stop=(j == 7),
                                     tile_position=(0, 0))
                    nc.tensor.matmul(po[64:128, 512 * t:512 * (t + 1)],
                                     vT[:, j, 128 * m + 64:128 * (m + 1)],
                                     a1[:, 512 * t:512 * (t + 1)],
                                     start=(j == 0), stop=(j == 7),
                                     tile_position=(0, 64))

            def conv1x1_gen():
                for c4 in range(4):
                    py = ps.tile([128, 1024], f32, tag="mmp", bufs=4,
                                 name=f"py{c4}")
                    for t in range(2):
                        for cb in range(4):
                            nc.tensor.matmul(py[:, 512 * t:512 * (t + 1)],
                                             wot[:, cb, 128 * c4:128 * (c4 + 1)],
                                             att[:, cb, 512 * t:512 * (t + 1)],
                                             start=(cb == 0), stop=(cb == 3))
                    yield None
                    nc.vector.tensor_scalar(
                        out=yt[:, c4, :], in0=py[:],
                        scalar1=1.0, scalar2=None,
                        op0=ALU.mult, op1=ALU.add,
                        accum_out=part[:, c4:c4 + 1])
                    bscr = tp.tile([128, 1024], bf, tag="bscr", bufs=2,
                                   name=f"bscr{c4}")
                    nc.scalar.activation(out=bscr[:], in_=py[:], func=AF.Square,
                                         accum_out=part[:, 4 + c4:5 + c4])
                    yield None

            def drain(g):
                if g is not None:
                    for _ in g:
                        pass

            def chain(*gens):
                for g in gens:
                    yield from g

            # ---- emission plan ----
            # Main conv chain [v8..v11, q0..q3, k4, k5] with each epilogue
            # woven into the following block (baseline discipline — keeps the
            # "mmp" PSUM ring at <= 4 live tiles). The fixes over the
            # baseline: k5's post no longer drains bare (it rotates with att0
            # inside conv6), post6/post7 are covered by attention weaves, and
            # att pairs are chained behind the k-posts they read (tensor
            # queue is FIFO, so emitting a dependent qk matmul ahead of its
            # producer's bcast matmul would deadlock the queue).
            fillers = []

            def pull_filler():
                while fillers:
                    g = fillers[0]
                    try:
                        next(g)
                        fillers.append(fillers.pop(0))
                        return
                    except StopIteration:
                        fillers.pop(0)

            def conv_block(cob):
                g = conv_gen(cob)
                pq_raw = next(g)
                first = True
                for _ in g:
                    if not first:
                        pull_filler()
                    first = False
                return pq_raw

            for ib, cob in enumerate([8, 9, 10, 11, 0, 1, 2, 3, 4, 5]):
                pq, raw = conv_block(cob)
                fillers.append(post_gen(cob, pq, raw))
                if ib == 0:
                    emit_warm_ar()
            fillers.append(att_gen(0))
            pq, raw = conv_block(6)
            fillers.append(chain(post_gen(6, pq, raw), att_gen(1)))
            pq, raw = conv_block(7)
            fillers.append(chain(post_gen(7, pq, raw), att_gen(2), att_gen(3)))
            # free-run the attention bulk (dense qk/out matmul streams), then
            # conv1x1 (needs all four pairs' outputs, so it cannot rotate).
            while fillers:
                pull_filler()
            drain(conv1x1_gen())

            # ---- BatchNorm: AllReduce 4KB of partial sums, then apply ----
            cin_d = dram.tile([128, 8], f32)
            cout_d = dram.tile([128, 8], f32)
            nc.gpsimd.dma_start(cin_d[:], part[:])
            nc.gpsimd.collective_compute(
                "AllReduce", ALU.add,
                ins=[cin_d[:].opt()], outs=[cout_d[:].opt()],
                replica_groups=[list(range(NCORES))])
            nc.sync.dma_start(stats[:], cout_d[:])

            var = sb.tile([128, 4], f32, tag="var")
            stdt = sb.tile([128, 4], f32, tag="stdt")
            rstd = sb.tile([128, 4], f32, tag="rstd")
            scl = sb.tile([128, 4], f32, tag="scl")
            sht = sb.tile([128, 4], f32, tag="sht")
            msq = sb.tile([128, 4], f32, tag="msq")
            tmp = sb.tile([128, 4], f32, tag="tmp")
            NINV = 1.0 / 8192.0
            nc.vector.tensor_scalar_mul(stats[:], stats[:], NINV)
            mean = stats[:, 0:4]
            ex2 = stats[:, 4:8]
            nc.vector.tensor_mul(msq[:], mean[:], mean[:])
            nc.vector.tensor_sub(var[:], ex2[:], msq[:])
            nc.scalar.activation(out=stdt[:], in_=var[:], func=AF.Sqrt,
                                 bias=epst[:], scale=1.0)
            nc.vector.reciprocal_approx_fast(out=rstd[:], in_=stdt[:])
            nc.vector.tensor_mul(scl[:], gbt[:, 0:4], rstd[:])
            nc.vector.tensor_mul(tmp[:], mean[:], scl[:])
            nc.vector.tensor_sub(sht[:], gbt[:, 4:8], tmp[:])
            # BN apply + ReLU on 512-column halves, split across three
            # engines (scalar's fused activation is ~2x a DVE 2-op chain, so
            # it takes 4 of the 8 halves), each half DMA'd out as soon as its
            # engine finishes it, spread over four queues.
            dma_q = [nc.sync, nc.gpsimd, nc.scalar, nc.sync]
            qi = 0
            for c4 in range(4):
                for h in range(2):
                    sl = slice(512 * h, 512 * (h + 1))
                    seg = yt[:, c4, sl]
                    if h == 0:
                        nc.scalar.activation(out=seg, in_=seg, func=AF.Relu,
                                             scale=scl[:, c4:c4 + 1],
                                             bias=sht[:, c4:c4 + 1])
                    else:
                        eng = nc.vector if c4 % 2 == 0 else nc.gpsimd
                        eng.tensor_scalar(out=seg, in0=seg,
                                          scalar1=scl[:, c4:c4 + 1],
                                          scalar2=sht[:, c4:c4 + 1],
                                          op0=ALU.mult, op1=ALU.add)
                        eng.tensor_scalar_max(out=seg, in0=seg, scalar1=0.0)
                    dma_q[qi % 4].dma_start(
                        out[128 * c4:128 * (c4 + 1), sl], seg)
                    qi += 1

    nc.compile()
    return nc


def _prep_inputs(x, W_qkv, W_out, gamma, beta):
    x = np.asarray(x, np.float32)
    W_qkv = np.asarray(W_qkv, np.float32)
    W_out = np.asarray(W_out, np.float32)
    gamma = np.asarray(gamma, np.float32)
    beta = np.asarray(beta, np.float32)

    xs = x.reshape(8, 4, 128, 32, 32)
    xpad = np.zeros((8, 4, 128, 34, 34), np.float32)
    xpad[:, :, :, 1:33, 1:33] = xs
    xpad = xpad.astype(BF)

    wqk = np.ascontiguousarray(
        W_qkv.reshape(12, 128, 4, 128, 3, 3)
        .transpose(0, 2, 3, 4, 5, 1).astype(BF))
    wo = np.ascontiguousarray(
        W_out[:, :, 0, 0].T.reshape(4, 128, 512).astype(BF))
    gb = np.ascontiguousarray(np.concatenate(
        [gamma.reshape(4, 128).T, beta.reshape(4, 128).T], axis=1)
        .astype(np.float32))
    p = np.arange(128)
    ones2 = np.ascontiguousarray(
        np.stack([p < 64, p >= 64], axis=1).astype(BF))
    sel2 = np.ascontiguousarray(
        np.stack([p < 64, p >= 64], axis=0).astype(np.float32))
    identv = np.eye(128, dtype=BF)

    common = {"wqk": wqk, "wo": wo, "gb": gb,
              "ones2": ones2, "sel2": sel2, "ident": identv}
    return [{"xpad": np.ascontiguousarray(xpad[b]), **common}
            for b in range(8)]


def kernel(x, W_qkv, W_out, gamma, beta):
    global _NC, LAST_RESULT
    if _NC is None:
        _NC = _build()
    in_maps = _prep_inputs(x, W_qkv, W_out, gamma, beta)
    res = bass_utils.run_bass_kernel_spmd(
        _NC, in_maps, core_ids=list(range(NCORES)))
    LAST_RESULT = res
    outs = [res.results[b]["out"].reshape(512, 32, 32) for b in range(8)]
    return np.stack(outs).astype(np.float32)



# revision 15
# speedup vs baseline: 1.3212x; 1.3212x over previous
"""Trainium2 Bass kernel for nn_Attention_73254962200646 (Winograd version).

Reference computation (per batch element b, all shapes hardcoded):
  qkv = conv3x3(x, W_qkv, pad=1)            x:[8,512,32,32], W_qkv:[1536,512,3,3]
  q,k,v -> [g=8 heads, n=1024, d=64]
  attn  = (q @ k^T) / (|q| |k| + eps)       cosine-similarity attention
  out   = attn @ v -> [512,32,32]
  out   = conv1x1(out, W_out); BatchNorm2d (batch stats); ReLU

Distribution: data-parallel over batch B=8 across the 8 NeuronCores (one
image per core). All compute core-local in bf16 (fp32 PSUM accumulation);
the only collective is a 4KB AllReduce of the BatchNorm partial sums.

The conv3x3 uses Winograd F(2x2,3x3): weights are transformed to U = G g G^T
on the host; the input transform V = B^T d B (all +-1 coefficients) runs on
the vector engine; the per-position products M_p = sum_ci U_p V_p are 16x4
matmuls of [128,128]x[128,256] per output-channel block (2.25x fewer PE
cycles than direct conv); the inverse transform Y = A^T M A runs on the
vector engine in two batched stages. Spatial pixels live in a tiled order
n = (2x+y)*256 + 16i + j throughout (attention and BN are permutation-
invariant over pixels); the final output DMA un-permutes to row-major.
"""

import numpy as np
import ml_dtypes

import concourse.tile as tile
import concourse.mybir as mybir
from concourse import bacc, bass_utils

BF = ml_dtypes.bfloat16
SMOOTH = 1e-4
BN_EPS = 1e-5
NCORES = 8

_NC = None
LAST_RESULT = None


def _build():
    f32 = mybir.dt.float32
    bf = mybir.dt.bfloat16
    AF = mybir.ActivationFunctionType
    ALU = mybir.AluOpType

    nc = bacc.Bacc("TRN2", target_bir_lowering=False, debug=False,
                   num_devices=NCORES)
    xin = nc.dram_tensor("xpad", [4, 128, 34, 34], bf, kind="ExternalInput").ap()
    wu = nc.dram_tensor("wu", [12, 4, 128, 4, 4, 128], bf,
                        kind="ExternalInput").ap()
    wo = nc.dram_tensor("wo", [4, 128, 512], bf, kind="ExternalInput").ap()
    gb = nc.dram_tensor("gb", [128, 8], f32, kind="ExternalInput").ap()
    ones2 = nc.dram_tensor("ones2", [128, 2], bf, kind="ExternalInput").ap()
    sel2 = nc.dram_tensor("sel2", [2, 128], f32, kind="ExternalInput").ap()
    ident = nc.dram_tensor("ident", [128, 128], bf, kind="ExternalInput").ap()
    out = nc.dram_tensor("out", [512, 1024], f32, kind="ExternalOutput").ap()

    with tile.TileContext(nc) as tc:
        with tc.tile_pool(name="sb", bufs=1) as sb, \
             tc.tile_pool(name="tp", bufs=2) as tp, \
             tc.tile_pool(name="ps", bufs=6, space="PSUM") as ps, \
             tc.tile_pool(name="dram", bufs=1, space="DRAM") as dram:

            xps = [sb.tile([128, 34, 34], bf, tag=f"xp{cb}", name=f"xp{cb}")
                   for cb in range(4)]
            vt = [sb.tile([128, 16, 256], bf, tag=f"vt{cb}", name=f"vt{cb}")
                  for cb in range(4)]
            identt = sb.tile([128, 128], bf, tag="identt")
            wot = sb.tile([128, 4, 512], bf, tag="wot")
            gbt = sb.tile([128, 8], f32, tag="gbt")
            ones2t = sb.tile([128, 2], bf, tag="ones2t")
            sel2t = sb.tile([2, 128], f32, tag="sel2t")
            qhat = sb.tile([128, 4, 1024], bf, tag="qhat")
            khat = sb.tile([128, 4, 1024], bf, tag="khat")
            vT = sb.tile([128, 8, 512], bf, tag="vT")
            att = sb.tile([128, 4, 1024], bf, tag="att")
            yt = sb.tile([128, 4, 1024], f32, tag="yt")
            part = sb.tile([128, 16], f32, tag="part")
            stats = sb.tile([128, 16], f32, tag="stats")
            epst = sb.tile([128, 1], f32, tag="epst")
            smt = sb.tile([2, 1], f32, tag="smt")

            # startup DMAs: sync queue is reserved for the weight stream.
            nc.scalar.dma_start(xps[0][:], xin[0])
            nc.gpsimd.dma_start(xps[1][:], xin[1])
            nc.scalar.dma_start(xps[2][:], xin[2])
            nc.gpsimd.dma_start(xps[3][:], xin[3])
            nc.gpsimd.dma_start(identt[:], ident)
            nc.gpsimd.dma_start(ones2t[:], ones2)
            nc.gpsimd.dma_start(sel2t[:], sel2)
            for cb in range(4):
                nc.gpsimd.dma_start(wot[:, cb], wo[cb])
            nc.gpsimd.dma_start(gbt[:], gb)
            nc.vector.memset(epst[:], BN_EPS)
            nc.vector.memset(smt[:], SMOOTH)

            def emit_v_transform():
                """V = B^T d B on the vector engine, k-major so vt[:, p]
                lands in the order the matmuls consume (p = 4k + c)."""
                for k in range(4):
                    tks = []
                    for cb in range(4):
                        tk = tp.tile([128, 16, 34], bf, tag="tkw", bufs=4,
                                     name=f"tk{k}_{cb}")
                        x = xps[cb]
                        r2 = lambda lo: x[:, lo:lo + 32].rearrange(
                            "p (i two) c -> p i two c", two=2)
                        ra = r2(0)[:, :, 0]    # rows 0,2..30
                        rb = r2(1)[:, :, 0]    # rows 1,3..31
                        rc = r2(2)[:, :, 0]    # rows 2,4..32
                        rd = r2(2)[:, :, 1]    # rows 3,5..33
                        if k == 0:
                            nc.vector.tensor_sub(tk[:], ra, rc)
                        elif k == 1:
                            nc.vector.tensor_add(tk[:], rb, rc)
                        elif k == 2:
                            nc.vector.tensor_sub(tk[:], rc, rb)
                        else:
                            nc.vector.tensor_sub(tk[:], rb, rd)
                        tks.append(tk)
                    for c in range(4):
                        for cb in range(4):
                            t = tks[cb]
                            c2 = lambda lo: t[:, :, lo:lo + 32].rearrange(
                                "p i (j two) -> p i j two", two=2)
                            ca = c2(0)[:, :, :, 0]
                            cbv = c2(1)[:, :, :, 0]
                            cc = c2(2)[:, :, :, 0]
                            cd = c2(2)[:, :, :, 1]
                            dst = vt[cb][:, 4 * k + c]
                            dst = dst.rearrange("p (i j) -> p i j", i=16)
                            if c == 0:
                                nc.vector.tensor_sub(dst, ca, cc)
                            elif c == 1:
                                nc.vector.tensor_add(dst, cbv, cc)
                            elif c == 2:
                                nc.vector.tensor_sub(dst, cc, cbv)
                            else:
                                nc.vector.tensor_sub(dst, cbv, cd)

            def emit_warm_ar():
                warm_in = dram.tile([1, 8], f32, name="warm_in")
                warm_out = dram.tile([1, 8], f32, name="warm_out")
                warm_sb = sb.tile([1, 8], f32, tag="warm_sb")
                nc.vector.memset(warm_sb[:], 0.0)
                nc.gpsimd.dma_start(warm_in[:], warm_sb[:])
                nc.gpsimd.collective_compute(
                    "AllReduce", ALU.add,
                    ins=[warm_in[:].opt()], outs=[warm_out[:].opt()],
                    replica_groups=[list(range(NCORES))])

            def wino_gen(cob):
                """Winograd conv block: 64 matmuls of [128,128]x[128,256]
                accumulating M_p over cin, scalar-evacuated to SBUF, then the
                inverse transform Y = A^T M A on the vector engine -> raw."""
                wuts = [tp.tile([128, 4, 4, 128], bf, tag="wu", bufs=6,
                                name=f"wu{cob}_{c}") for c in range(4)]
                for c in range(4):
                    nc.sync.dma_start(wuts[c][:], wu[cob, c])
                raw = tp.tile([128, 1024], bf, tag="raw", bufs=4,
                              name=f"raw{cob}")
                msb = tp.tile([128, 16, 256], bf, tag="msb", bufs=2,
                              name=f"msb{cob}")
                yield raw
                for p in range(16):
                    mm = ps.tile([128, 256], f32, tag="wps", bufs=2,
                                 name=f"mm{cob}_{p}")
                    for cb in range(4):
                        nc.tensor.matmul(mm[:], wuts[p // 4][:, p % 4, cb],
                                         vt[cb][:, p],
                                         start=(cb == 0), stop=(cb == 3))
                    nc.scalar.copy(msb[:, p], mm[:])
                    if p % 2 == 1:
                        yield None
                # inverse transform: stage1 P/Q, stage2 -> raw pixel classes
                pqt_ = tp.tile([128, 8, 256], bf, tag="pqt", bufs=2,
                               name=f"pqt{cob}")
                tw = tp.tile([128, 4, 256], bf, tag="wtmp", bufs=2,
                             name=f"tw{cob}")
                nc.vector.tensor_add(tw[:], msb[:, 0:4], msb[:, 4:8])
                nc.vector.tensor_add(pqt_[:, 0:4], tw[:], msb[:, 8:12])
                nc.vector.tensor_sub(tw[:], msb[:, 4:8], msb[:, 8:12])
                nc.vector.tensor_sub(pqt_[:, 4:8], tw[:], msb[:, 12:16])
                yield None
                s = pqt_[:].rearrange("p (g k) (i j) -> p g k i j",
                                      g=2, i=16)
                # raw is row-major over pixels: n = 64i + 32x + 2j + y, and
                # the P/Q row (g) of stage1 is the output-row parity x.
                rv = raw.rearrange("p (i x j y) -> p x y i j",
                                   i=16, x=2, j=16, y=2)
                tw2 = tp.tile([128, 2, 256], bf, tag="wtmp2", bufs=2,
                              name=f"tw2{cob}")
                t2 = tw2[:].rearrange("p g (i j) -> p g i j", i=16)
                nc.vector.tensor_add(t2, s[:, :, 0], s[:, :, 1])
                nc.vector.tensor_add(rv[:, :, 0], t2, s[:, :, 2])
                nc.vector.tensor_sub(t2, s[:, :, 1], s[:, :, 2])
                nc.vector.tensor_sub(rv[:, :, 1], t2, s[:, :, 3])
                yield None

            def post_gen(cob, raw):
                """Per-kind epilogue consuming a conv block's raw output."""
                if cob >= 8:   # v block: PE-transpose into vT
                    m = cob - 8
                    for c2 in range(2):
                        pt = ps.tile([128, 512], bf, tag="patt", bufs=4,
                                     name=f"pt{cob}_{c2}")
                        for c in range(4):
                            j = 4 * c2 + c
                            nc.tensor.transpose(pt[:, 128 * c:128 * (c + 1)],
                                                raw[:, 128 * j:128 * (j + 1)],
                                                identt[:])
                        dstv = vT[:, 4 * c2:4 * (c2 + 1), 128 * m:128 * (m + 1)]
                        srcv = pt[:].rearrange("p (a b) -> p a b", a=4)
                        if c2 == 0:
                            nc.scalar.copy(dstv, srcv)
                        else:
                            nc.vector.tensor_copy(out=dstv, in_=srcv)
                        yield None
                else:          # q/k block: cosine norms + normalized copy
                    m = cob % 4
                    dst = qhat if cob < 4 else khat
                    nrm = tp.tile([2, 1024], f32, tag="nrm", bufs=2,
                                  name=f"nrm{cob}")
                    inv = tp.tile([2, 1024], f32, tag="inv", bufs=2,
                                  name=f"inv{cob}")
                    sq = tp.tile([128, 1024], bf, tag="sq", bufs=2,
                                 name=f"sq{cob}")
                    nc.scalar.square(sq[:, 0:512], raw[:, 0:512])
                    nc.scalar.square(sq[:, 512:1024], raw[:, 512:1024])
                    yield None
                    for t in range(2):
                        pss = ps.tile([2, 512], f32, tag="patt", bufs=4,
                                      name=f"pss{cob}_{t}")
                        nc.tensor.matmul(pss[:], ones2t[:],
                                         sq[:, 512 * t:512 * (t + 1)],
                                         start=True, stop=True)
                        nc.scalar.activation(out=nrm[:, 512 * t:512 * (t + 1)],
                                             in_=pss[:], func=AF.Sqrt,
                                             bias=smt[:], scale=1.0)
                        yield None
                    nc.vector.reciprocal_approx_fast(out=inv[:], in_=nrm[:])
                    yield None
                    for t in range(2):
                        pbc = ps.tile([128, 512], f32, tag="patt", bufs=4,
                                      name=f"pbc{cob}_{t}")
                        nc.tensor.matmul(pbc[:], sel2t[:],
                                         inv[:, 512 * t:512 * (t + 1)],
                                         start=True, stop=True)
                        nc.vector.tensor_mul(dst[:, m, 512 * t:512 * (t + 1)],
                                             raw[:, 512 * t:512 * (t + 1)],
                                             pbc[:])
                        yield None

            def att_gen(m):
                """Attention pair (heads 2m, 2m+1), PSUM in 512-wide halves."""
                pot = [ps.tile([128, 512], f32, tag="pacc", bufs=2,
                               name=f"po{m}_{t}") for t in range(2)]
                prev = None
                for j in range(8):
                    if prev is not None:
                        emit_outT(m, pot, *prev)
                    pas = []
                    for h in range(2):
                        for t in range(2):
                            pa = ps.tile([128, 512], f32, tag="patt", bufs=4,
                                         name=f"pa{m}_{j}_{h}_{t}")
                            nc.tensor.matmul(
                                pa[:],
                                khat[64 * h:64 * (h + 1), m,
                                     128 * j:128 * (j + 1)],
                                qhat[64 * h:64 * (h + 1), m,
                                     512 * t:512 * (t + 1)],
                                start=True, stop=True)
                            pas.append(pa)
                    a0 = tp.tile([128, 1024], bf, tag="attnT", bufs=6,
                                 name=f"a0_{m}_{j}")
                    a1 = tp.tile([128, 1024], bf, tag="attnT", bufs=6,
                                 name=f"a1_{m}_{j}")
                    nc.scalar.copy(a0[:, 0:512], pas[0][:])
                    nc.vector.tensor_copy(out=a0[:, 512:1024], in_=pas[1][:])
                    nc.scalar.copy(a1[:, 0:512], pas[2][:])
                    nc.vector.tensor_copy(out=a1[:, 512:1024], in_=pas[3][:])
                    prev = (j, a0, a1)
                    yield None
                emit_outT(m, pot, *prev)
                for t in range(2):
                    sl = slice(512 * t, 512 * (t + 1))
                    if t == 0:
                        nc.scalar.copy(att[:, m, sl], pot[t][:])
                    else:
                        nc.vector.tensor_copy(out=att[:, m, sl], in_=pot[t][:])
                yield None

            def emit_outT(m, pot, j, a0, a1):
                for t in range(2):
                    nc.tensor.matmul(pot[t][0:64, :],
                                     vT[:, j, 128 * m:128 * m + 64],
                                     a0[:, 512 * t:512 * (t + 1)],
                                     start=(j == 0), stop=(j == 7),
                                     tile_position=(0, 0))
                    nc.tensor.matmul(pot[t][64:128, :],
                                     vT[:, j, 128 * m + 64:128 * (m + 1)],
                                     a1[:, 512 * t:512 * (t + 1)],
                                     start=(j == 0), stop=(j == 7),
                                     tile_position=(0, 64))

            def conv1x1_gen():
                for c4 in range(4):
                    pys = []
                    for t in range(2):
                        py = ps.tile([128, 512], f32, tag="pacc", bufs=2,
                                     name=f"py{c4}_{t}")
                        for cb in range(4):
                            nc.tensor.matmul(py[:],
                                             wot[:, cb, 128 * c4:128 * (c4 + 1)],
                                             att[:, cb, 512 * t:512 * (t + 1)],
                                             start=(cb == 0), stop=(cb == 3))
                        pys.append(py)
                    yield None
                    ytv = yt[:, c4, :].rearrange("p (t f) -> p t f", t=2)
                    nc.vector.tensor_scalar(
                        out=ytv[:, 0], in0=pys[0][:],
                        scalar1=1.0, scalar2=None,
                        op0=ALU.mult, op1=ALU.add,
                        accum_out=part[:, 2 * c4:2 * c4 + 1])
                    bscr = tp.tile([128, 1024], bf, tag="bscr", bufs=2,
                                   name=f"bscr{c4}")
                    nc.scalar.activation(out=bscr[:, 0:512], in_=pys[0][:],
                                         func=AF.Square,
                                         accum_out=part[:, 8 + 2 * c4:
                                                        9 + 2 * c4])
                    yield None
                    nc.vector.tensor_scalar(
                        out=ytv[:, 1], in0=pys[1][:],
                        scalar1=1.0, scalar2=None,
                        op0=ALU.mult, op1=ALU.add,
                        accum_out=part[:, 2 * c4 + 1:2 * c4 + 2])
                    nc.scalar.activation(out=bscr[:, 512:1024], in_=pys[1][:],
                                         func=AF.Square,
                                         accum_out=part[:, 9 + 2 * c4:
                                                        10 + 2 * c4])
                    yield None

            def drain(g):
                if g is not None:
                    for _ in g:
                        pass

            def chain(*gens):
                for g in gens:
                    yield from g

            # ---- emission plan ----
            # V transform first (vector queue fills while input DMAs land),
            # then the 12 Winograd blocks [v8..v11, q0..q3, k4..k7] with each
            # epilogue woven into the following block via the filler
            # rotation. Attention pairs run as ONE serial chain (a single
            # [128,512]-granular PSUM ring serves pa/po/pss/pbc/pt/py: only
            # one pair's accumulators are ever live). Emission-order rule
            # (tensor queue is FIFO): att pair m's qk matmuls are emitted
            # after post(k_m)'s bcast matmuls by chain position.
            emit_v_transform()

            fillers = []

            def pull_filler():
                while fillers:
                    g = fillers[0]
                    try:
                        next(g)
                        fillers.append(fillers.pop(0))
                        return
                    except StopIteration:
                        fillers.pop(0)

            for ib, cob in enumerate([8, 9, 10, 11, 0, 1, 2, 3, 4, 5, 6, 7]):
                g = wino_gen(cob)
                raw = next(g)
                first = True
                for _ in g:
                    if not first:
                        pull_filler()
                    first = False
                fillers.append(post_gen(cob, raw))
                if ib == 0:
                    emit_warm_ar()
                if cob == 5:
                    fillers.append(chain(att_gen(0), att_gen(1), att_gen(2),
                                         att_gen(3)))
            while fillers:
                pull_filler()
            drain(conv1x1_gen())

            # ---- BatchNorm: AllReduce 4KB of partial sums, then apply ----
            cin_d = dram.tile([128, 16], f32)
            cout_d = dram.tile([128, 16], f32)
            nc.gpsimd.dma_start(cin_d[:], part[:])
            nc.gpsimd.collective_compute(
                "AllReduce", ALU.add,
                ins=[cin_d[:].opt()], outs=[cout_d[:].opt()],
                replica_groups=[list(range(NCORES))])
            nc.sync.dma_start(stats[:], cout_d[:])

            var = sb.tile([128, 4], f32, tag="var")
            stdt = sb.tile([128, 4], f32, tag="stdt")
            rstd = sb.tile([128, 4], f32, tag="rstd")
            scl = sb.tile([128, 4], f32, tag="scl")
            sht = sb.tile([128, 4], f32, tag="sht")
            msq = sb.tile([128, 4], f32, tag="msq")
            tmp = sb.tile([128, 4], f32, tag="tmp")
            comb = sb.tile([128, 8], f32, tag="comb")
            NINV = 1.0 / 8192.0
            nc.vector.tensor_scalar_mul(stats[:], stats[:], NINV)
            pairv = stats[:].rearrange("p (a c two) -> p a c two",
                                       a=2, two=2)
            combv = comb[:].rearrange("p (a c) -> p a c", a=2)
            nc.vector.tensor_add(combv, pairv[:, :, :, 0], pairv[:, :, :, 1])
            mean = comb[:, 0:4]
            ex2 = comb[:, 4:8]
            nc.vector.tensor_mul(msq[:], mean[:], mean[:])
            nc.vector.tensor_sub(var[:], ex2[:], msq[:])
            nc.scalar.activation(out=stdt[:], in_=var[:], func=AF.Sqrt,
                                 bias=epst[:], scale=1.0)
            nc.vector.reciprocal_approx_fast(out=rstd[:], in_=stdt[:])
            nc.vector.tensor_mul(scl[:], gbt[:, 0:4], rstd[:])
            nc.vector.tensor_mul(tmp[:], mean[:], scl[:])
            nc.vector.tensor_sub(sht[:], gbt[:, 4:8], tmp[:])

            # BN apply + ReLU on 512-column halves split scalar/vector, each
            # half DMA'd out (un-permuting the tiled pixel order) as soon as
            # its engine finishes.
            dma_q = [nc.sync, nc.gpsimd, nc.scalar, nc.sync]
            qi = 0
            for c4 in range(4):
                for h in range(2):
                    sl = slice(512 * h, 512 * (h + 1))
                    seg = yt[:, c4, sl]
                    if h == 0:
                        nc.scalar.activation(out=seg, in_=seg, func=AF.Relu,
                                             scale=scl[:, c4:c4 + 1],
                                             bias=sht[:, c4:c4 + 1])
                    else:
                        nc.vector.tensor_scalar(out=seg, in0=seg,
                                                scalar1=scl[:, c4:c4 + 1],
                                                scalar2=sht[:, c4:c4 + 1],
                                                op0=ALU.mult, op1=ALU.add)
                        nc.vector.tensor_scalar_max(out=seg, in0=seg,
                                                    scalar1=0.0)
                    dma_q[qi % 4].dma_start(
                        out[128 * c4:128 * (c4 + 1), sl], seg)
                    qi += 1

    nc.compile()
    return nc


def _prep_inputs(x, W_qkv, W_out, gamma, beta):
    x = np.asarray(x, np.float32)
    W_qkv = np.asarray(W_qkv, np.float32)
    W_out = np.asarray(W_out, np.float32)
    gamma = np.asarray(gamma, np.float32)
    beta = np.asarray(beta, np.float32)

    xs = x.reshape(8, 4, 128, 32, 32)
    xpad = np.zeros((8, 4, 128, 34, 34), np.float32)
    xpad[:, :, :, 1:33, 1:33] = xs
    xpad = xpad.astype(BF)

    G = np.array([[1, 0, 0], [.5, .5, .5], [.5, -.5, .5], [0, 0, 1]],
                 np.float32)
    U4 = np.einsum('ru,oiuv,cv->rcoi', G, W_qkv, G)     # [4,4,1536,512]
    t = U4.reshape(4, 4, 12, 128, 4, 128)               # r c cob co cb ci
    wu = np.ascontiguousarray(
        t.transpose(2, 0, 5, 1, 4, 3).astype(BF))       # cob r ci c cb co

    wo = np.ascontiguousarray(
        W_out[:, :, 0, 0].T.reshape(4, 128, 512).astype(BF))
    gb = np.ascontiguousarray(np.concatenate(
        [gamma.reshape(4, 128).T, beta.reshape(4, 128).T], axis=1)
        .astype(np.float32))
    p = np.arange(128)
    ones2 = np.ascontiguousarray(
        np.stack([p < 64, p >= 64], axis=1).astype(BF))
    sel2 = np.ascontiguousarray(
        np.stack([p < 64, p >= 64], axis=0).astype(np.float32))
    identv = np.eye(128, dtype=BF)

    common = {"wu": wu, "wo": wo, "gb": gb,
              "ones2": ones2, "sel2": sel2, "ident": identv}
    return [{"xpad": np.ascontiguousarray(xpad[b]), **common}
            for b in range(8)]


def kernel(x, W_qkv, W_out, gamma, beta):
    global _NC, LAST_RESULT
    if _NC is None:
        _NC = _build()
    in_maps = _prep_inputs(x, W_qkv, W_out, gamma, beta)
    res = bass_utils.run_bass_kernel_spmd(
        _NC, in_maps, core_ids=list(range(NCORES)))
    LAST_RESULT = res
    outs = [res.results[b]["out"].reshape(512, 32, 32) for b in range(8)]
    return np.stack(outs).astype(np.float32)
